# revision 5
# baseline (speedup 1.0000x reference)
"""Trainium2 Bass kernel for the dynamic-attention-block CNN (nn_DAB).

Data-parallel over batch: 8 samples -> 8 NeuronCores. Each core runs the
full per-sample network with activations resident in SBUF as padded
"frames": 128 partitions = 64 channels x 2 image halves, each half a
98x196 zero-padded row-major frame (rows -1..96 / 95..192 of the 192x192
image, cols -2..193).

Conv structure (all single 128-partition matmuls; the two image halves
ride in one instruction via block-diagonal weights):
  - 3x3 convs and dynamic depthwise convs run in fp8e4m3 DoubleRow mode:
    taps are processed in pairs (lhsT [128,2,128], rhs [128,2,N] with the
    pair dim striding between the two tap offsets), 5 passes per conv.
    Weights are pre-scaled by 16 (convs) / 64 (dw) to dodge fp8
    subnormals; the inverse scale is folded into the scalar-engine
    activation that drains PSUM.
  - 1x1 convs + channel-attention gates run in bf16: the x*att residual
    is an extra diagonal-matrix matmul accumulating into the same PSUM
    group, so no vector-engine gating pass exists at all.
  - The additive 32x32-upsampled map is folded into conv2 as one K=18
    matmul pass: 18 partitions hold the 9 tap-shifted copies of the
    upsampled map for each half, weights are the channel-summed conv2
    taps.
Activation outputs are written by the scalar engine directly in the
dtype the consumer needs (fp8 frame for the next conv, bf16 where the
gate needs precision). Residual add + output happens on DVE from f32
x+b3 staged via DMA.
"""

import sys

for _p in ("/opt/trn_rl_repo", "/root/.axon_site/_ro/pypackages"):
    if _p not in sys.path:
        sys.path.insert(0, _p)

import numpy as np
import ml_dtypes

BF16 = ml_dtypes.bfloat16
F8 = ml_dtypes.float8_e4m3

B, C, H, W = 8, 64, 192, 192
HW = H * W
FR, FC = 98, 196          # frame rows / cols per half
FF = FR * FC              # frame elems per partition
Q0 = 1 * FC + 2           # first interior frame position (row 1, col 2)
QL = 96 * FC + 194 - Q0   # sweep length covering all interior rows
TILE = 512
ALPHA = 0.1               # leaky slope
WS = 16.0                 # fp8 conv weight pre-scale
KS = 64.0                 # fp8 dw kernel pre-scale

# bias pack columns
BI_B1, BI_B2, BI_CB1, BI_CB2, BI_Z = range(5)

# DoubleRow tap pairing: (tap_a, tap_b) with taps t = 3*dy + dx,
# delta(t) = (dy-1)*FC + (dx-1).  5 passes cover all 9 taps; the last
# pass's second slot has zero weight (stride 0 keeps the read in-bounds).
PAIRS = [(0, 1), (3, 4), (6, 7), (2, 5), (8, None)]

_CACHE = {}


def _delta(t):
    return (t // 3 - 1) * FC + (t % 3 - 1)


def _qtiles():
    out = []
    q = Q0
    while q < Q0 + QL:
        n = min(TILE, Q0 + QL - q)
        out.append((q, n))
        q += n
    return out


def _build_nc():
    import concourse.bacc as bacc
    import concourse.mybir as mybir
    from concourse import tile

    f32 = mybir.dt.float32
    bf16 = mybir.dt.bfloat16
    f8 = mybir.dt.float8e4
    AF = mybir.ActivationFunctionType
    ALU = mybir.AluOpType
    DR = mybir.MatmulPerfMode.DoubleRow

    nc = bacc.Bacc("TRN2", target_bir_lowering=False, debug=False, num_devices=8)

    xb_d = nc.dram_tensor("xb", [128, FF], bf16, kind="ExternalInput").ap()
    x8_d = nc.dram_tensor("x8", [128, FF], f8, kind="ExternalInput").ap()
    xr_d = nc.dram_tensor("xr", [C, HW], f32, kind="ExternalInput").ap()
    w1_d = nc.dram_tensor("w1", [128, 5, 2, 128], f8, kind="ExternalInput").ap()
    w2_d = nc.dram_tensor("w2", [128, 5, 2, 128], f8, kind="ExternalInput").ap()
    w3_d = nc.dram_tensor("w3", [128, 5, 2, 128], f8, kind="ExternalInput").ap()
    kd1_d = nc.dram_tensor("kd1", [128, 5, 2, 128], f8, kind="ExternalInput").ap()
    kd2_d = nc.dram_tensor("kd2", [128, 5, 2, 128], f8, kind="ExternalInput").ap()
    cw1_d = nc.dram_tensor("cw1", [128, 128], bf16, kind="ExternalInput").ap()
    cw2_d = nc.dram_tensor("cw2", [128, 128], bf16, kind="ExternalInput").ap()
    g1_d = nc.dram_tensor("g1", [128, 128], bf16, kind="ExternalInput").ap()
    g2_d = nc.dram_tensor("g2", [128, 128], bf16, kind="ExternalInput").ap()
    wa_d = nc.dram_tensor("wa", [18, 128], bf16, kind="ExternalInput").ap()
    af_d = nc.dram_tensor("af", [18, FF], bf16, kind="ExternalInput").ap()
    bias_d = nc.dram_tensor("bias", [128, 5], f32, kind="ExternalInput").ap()
    y_d = nc.dram_tensor("y", [C, HW], f32, kind="ExternalOutput").ap()

    qt = _qtiles()

    from contextlib import ExitStack
    with tile.TileContext(nc) as tc, ExitStack() as ctx:
        wpool = ctx.enter_context(tc.tile_pool(name="w", bufs=1))
        fbpool = ctx.enter_context(tc.tile_pool(name="fb", bufs=2))
        f8pool = ctx.enter_context(tc.tile_pool(name="f8", bufs=3))
        stg = ctx.enter_context(tc.tile_pool(name="stg", bufs=2))
        outp = ctx.enter_context(tc.tile_pool(name="outp", bufs=2))
        t1p = ctx.enter_context(tc.tile_pool(name="t1", bufs=4))
        psA = ctx.enter_context(tc.tile_pool(name="psA", bufs=4, space="PSUM"))
        psB = ctx.enter_context(tc.tile_pool(name="psB", bufs=2, space="PSUM"))

        # ---- weights / constants to SBUF ----
        w1 = wpool.tile([128, 5, 2, 128], f8, tag="w1")
        w2 = wpool.tile([128, 5, 2, 128], f8, tag="w2")
        w3 = wpool.tile([128, 5, 2, 128], f8, tag="w3")
        kd1 = wpool.tile([128, 5, 2, 128], f8, tag="kd1")
        kd2 = wpool.tile([128, 5, 2, 128], f8, tag="kd2")
        cw1 = wpool.tile([128, 128], bf16, tag="cw1")
        cw2 = wpool.tile([128, 128], bf16, tag="cw2")
        g1 = wpool.tile([128, 128], bf16, tag="g1")
        g2 = wpool.tile([128, 128], bf16, tag="g2")
        wa = wpool.tile([18, 128], bf16, tag="wa")
        af = wpool.tile([18, FF], bf16, tag="af")
        bias = wpool.tile([128, 5], f32, tag="bias")
        for t, d in ((w1, w1_d), (w2, w2_d), (w3, w3_d), (kd1, kd1_d),
                     (kd2, kd2_d), (cw1, cw1_d), (cw2, cw2_d), (g1, g1_d),
                     (g2, g2_d), (wa, wa_d), (bias, bias_d)):
            nc.gpsimd.dma_start(out=t[...], in_=d)
        nc.scalar.dma_start(out=af[:, :], in_=af_d)

        def cold(col):
            return bias[:, col:col + 1]

        # ---- input frames: host pre-padded, straight contiguous DMA ----
        Xb = fbpool.tile([128, FF], bf16, tag="fb")
        X8 = f8pool.tile([128, FF], f8, tag="f8")
        nchunk = 6
        step = (FF + nchunk - 1) // nchunk
        for k in range(nchunk):
            c0, c1 = k * step, min((k + 1) * step, FF)
            eng = nc.sync if k % 2 == 0 else nc.scalar
            eng.dma_start(out=Xb[:, c0:c1], in_=xb_d[:, c0:c1])
        step8 = (FF + 3) // 4
        for k in range(4):
            c0, c1 = k * step8, min((k + 1) * step8, FF)
            eng = nc.sync if k % 2 == 0 else nc.gpsimd
            eng.dma_start(out=X8[:, c0:c1], in_=x8_d[:, c0:c1])

        def pads_and_halo(m):
            mv = m[:, :].rearrange("p (a b) -> p a b", b=FC)
            nc.gpsimd.memset(mv[0:64, 0, :], 0.0)
            nc.gpsimd.memset(mv[64:128, FR - 1, :], 0.0)
            nc.gpsimd.memset(mv[:, :, 0:2], 0.0)
            nc.gpsimd.memset(mv[:, :, FC - 2:FC], 0.0)
            nc.gpsimd.dma_start(out=mv[0:64, FR - 1, :], in_=mv[64:128, 1, :])
            nc.gpsimd.dma_start(out=mv[64:128, 0, :], in_=mv[0:64, 96, :])

        def dr_rhs(m8, q, n, pair):
            ta, tb = pair
            base = q + _delta(ta)
            stride = 0 if tb is None else _delta(tb) - _delta(ta)
            r = m8[:, base:base + 1].copy()
            r.ap[1] = [stride, 2]
            r.ap.append([1, n])
            return r

        def conv_dr(ps, wsb, m8, q, n):
            for p in range(5):
                nc.tensor.matmul(ps[:, :n], wsb[:, p, :, :], dr_rhs(m8, q, n, PAIRS[p]),
                                 start=(p == 0), stop=(p == 4), perf_mode=DR,
                                 skip_group_check=True)

        def da_stage(inb, in8, kdsb, cwsb, gsb, cb_col, out8):
            for (q, n) in qt:
                pa = psA.tile([128, TILE], f32, tag="psA")
                conv_dr(pa, kdsb, in8, q, n)
                t1 = t1p.tile([128, TILE], bf16, tag="t1")
                nc.scalar.activation(t1[:, :n], pa[:, :n], AF.Prelu,
                                     scale=1.0 / KS, bias=cold(BI_Z), alpha=ALPHA)
                pb = psB.tile([128, TILE], f32, tag="psB")
                nc.tensor.matmul(pb[:, :n], cwsb[:, :], t1[:, :n],
                                 start=True, stop=False, skip_group_check=True)
                nc.tensor.matmul(pb[:, :n], gsb[:, :], inb[:, q:q + n],
                                 start=False, stop=True, skip_group_check=True)
                nc.scalar.activation(out8[:, q:q + n], pb[:, :n], AF.Prelu,
                                     bias=cold(cb_col), alpha=ALPHA)
            pads_and_halo(out8)

        # ---- network ----
        O1 = f8pool.tile([128, FF], f8, tag="f8")
        da_stage(Xb, X8, kd1, cw1, g1, BI_CB1, O1)

        # conv1 -> prelu -> fp8 frame
        O2 = f8pool.tile([128, FF], f8, tag="f8")
        for (q, n) in qt:
            pa = psA.tile([128, TILE], f32, tag="psA")
            conv_dr(pa, w1, O1, q, n)
            nc.scalar.activation(O2[:, q:q + n], pa[:, :n], AF.Prelu,
                                 scale=1.0 / WS, bias=cold(BI_B1), alpha=ALPHA)
        pads_and_halo(O2)

        # conv2 (+ additive map as a K=18 pass) -> identity+bias -> bf16 + fp8
        O3b = fbpool.tile([128, FF], bf16, tag="fb")
        O38 = f8pool.tile([128, FF], f8, tag="f8")
        for (q, n) in qt:
            pa = psA.tile([128, TILE], f32, tag="psA")
            for p in range(5):
                nc.tensor.matmul(pa[:, :n], w2[:, p, :, :], dr_rhs(O2, q, n, PAIRS[p]),
                                 start=(p == 0), stop=False, perf_mode=DR,
                                 skip_group_check=True)
            nc.tensor.matmul(pa[:, :n], wa[:, :], af[:, q:q + n],
                             start=False, stop=True, skip_group_check=True)
            nc.scalar.activation(O3b[:, q:q + n], pa[:, :n], AF.Identity,
                                 scale=1.0 / WS, bias=cold(BI_B2))
            nc.vector.tensor_copy(O38[:, q:q + n], O3b[:, q:q + n])
        pads_and_halo(O3b)
        pads_and_halo(O38)

        O4 = f8pool.tile([128, FF], f8, tag="f8")
        da_stage(O3b, O38, kd2, cw2, g2, BI_CB2, O4)

        # ---- conv3 fused with +b3 (host-baked into xr) and residual ----
        for k in range(12):
            xs = stg.tile([128, 8, 192], f32, tag="xs")
            nc.sync.dma_start(
                out=xs[0:64, :, :],
                in_=xr_d[:, 8 * k * 192:(8 * k + 8) * 192]
                .rearrange("p (r c) -> p r c", c=192))
            nc.sync.dma_start(
                out=xs[64:128, :, :],
                in_=xr_d[:, (96 + 8 * k) * 192:(96 + 8 * k + 8) * 192]
                .rearrange("p (r c) -> p r c", c=192))
            ot = outp.tile([128, 8, 192], f32, tag="ot")
            for m in range(4):
                r = 1 + 8 * k + 2 * m
                qr = r * FC + 2
                pa = psA.tile([128, TILE], f32, tag="psA")
                conv_dr(pa, w3, O4, qr, 388)
                nc.vector.scalar_tensor_tensor(
                    ot[:, 2 * m:2 * m + 2, :],
                    pa[:, 0:392].rearrange("p (a b) -> p a b", a=2)[:, :, 0:192],
                    1.0 / WS, xs[:, 2 * m:2 * m + 2, :],
                    op0=ALU.mult, op1=ALU.add)
            nc.sync.dma_start(
                out=y_d[:, 8 * k * 192:(8 * k + 8) * 192]
                .rearrange("p (r c) -> p r c", c=192),
                in_=ot[0:64, :, :])
            nc.sync.dma_start(
                out=y_d[:, (96 + 8 * k) * 192:(96 + 8 * k + 8) * 192]
                .rearrange("p (r c) -> p r c", c=192),
                in_=ot[64:128, :, :])

    nc.compile()
    return nc


def _pad_frame(xb, dtype):
    """(64,192,192) fp32 -> (128, FR*FC) dual-half padded frame."""
    fr = np.zeros((128, FR, FC), np.float32)
    fr[0:64, 1:97, 2:194] = xb[:, 0:96, :]
    fr[0:64, 97, 2:194] = xb[:, 96, :]
    fr[64:128, 1:97, 2:194] = xb[:, 96:192, :]
    fr[64:128, 0, 2:194] = xb[:, 95, :]
    return np.ascontiguousarray(fr.reshape(128, FF)).astype(dtype)


def _leaky_np(v):
    return np.where(v >= 0, v, ALPHA * v)


def _host_precompute(x, d, p):
    """Build per-core input maps. p: dict of raw weight arrays."""
    d = d.astype(np.float64)
    kern = {}
    att = {}
    for i in (1, 2):
        kw1, kw2 = p[f'da{i}_kw1'].astype(np.float64), p[f'da{i}_kw2'].astype(np.float64)
        ca1, ca2 = p[f'da{i}_ca1'].astype(np.float64), p[f'da{i}_ca2'].astype(np.float64)
        kern[i] = _leaky_np(d @ kw1.T) @ kw2.T          # (B, 576) [c*9+t]
        z = _leaky_np(d @ ca1.T) @ ca2.T
        att[i] = 1.0 / (1.0 + np.exp(-z))               # (B, 64)
    a32 = _leaky_np(d @ p['add_w1'].astype(np.float64).T) @ \
        p['add_w2'].astype(np.float64).T                # (B, 1024)

    cidx = np.arange(128) % 64
    hidx = np.arange(128) // 64

    def convw_dr(w):
        # (O, C, 3, 3) fp32 -> [128, 5, 2, 128] f8 block-diag DoubleRow taps
        wq = (w.astype(np.float32) * WS).astype(F8).astype(np.float32)
        wt = wq.transpose(1, 2, 3, 0).reshape(64, 9, 64)  # [c, t, o]
        out = np.zeros((128, 5, 2, 128), np.float32)
        for pi, (ta, tb) in enumerate(PAIRS):
            blk = np.zeros((64, 2, 64), np.float32)
            blk[:, 0, :] = wt[:, ta, :]
            if tb is not None:
                blk[:, 1, :] = wt[:, tb, :]
            out[0:64, pi, :, 0:64] = blk
            out[64:128, pi, :, 64:128] = blk
        return np.ascontiguousarray(out).astype(F8)

    def cw_bd(w):
        # (O, C) -> [128, 128] bf16 block-diag: [p, o]
        out = np.zeros((128, 128), np.float32)
        out[0:64, 0:64] = w.T
        out[64:128, 64:128] = w.T
        return np.ascontiguousarray(out).astype(BF16)

    w1 = convw_dr(p['conv1_w'])
    w2 = convw_dr(p['conv2_w'])
    w3 = convw_dr(p['conv3_w'])
    cw1 = cw_bd(p['da1_cw'])
    cw2 = cw_bd(p['da2_cw'])

    # additive-map conv weights: wa[(h,t), o_col] = WS * sum_c conv2_w[o,c,t]
    w2sum = p['conv2_w'].astype(np.float64).sum(axis=1).reshape(64, 9)  # [o, t]
    wa = np.zeros((18, 128), np.float32)
    for h in range(2):
        for t in range(9):
            wa[h * 9 + t, h * 64:(h + 1) * 64] = WS * w2sum[:, t]
    wa = np.ascontiguousarray(wa).astype(BF16)

    xr_all = x.astype(np.float32) + p['conv3_b'].astype(np.float32)[None, :, None, None]

    maps = []
    for b in range(B):
        kd = {}
        for i in (1, 2):
            kc = (kern[i][b].reshape(64, 9).astype(np.float32) * KS) \
                .astype(F8).astype(np.float32)           # [c, t]
            kdl = np.zeros((128, 5, 2, 128), np.float32)
            for pi, (ta, tb) in enumerate(PAIRS):
                kdl[np.arange(128), pi, 0, np.arange(128)] = kc[cidx, ta]
                if tb is not None:
                    kdl[np.arange(128), pi, 1, np.arange(128)] = kc[cidx, tb]
            kd[i] = np.ascontiguousarray(kdl).astype(F8)
        g = {i: np.ascontiguousarray(_diag128(att[i][b][cidx])).astype(BF16)
             for i in (1, 2)}
        bias = np.zeros((128, 5), np.float32)
        bias[:, BI_B1] = p['conv1_b'][cidx]
        bias[:, BI_B2] = p['conv2_b'][cidx]
        bias[:, BI_CB1] = p['da1_cb'][cidx]
        bias[:, BI_CB2] = p['da2_cb'][cidx]

        # additive map frames: 18 partitions = 2 halves x 9 tap shifts
        a = a32[b].astype(np.float32).reshape(32, 32)
        aup = a[np.arange(192) // 6][:, np.arange(192) // 6]  # (192,192)
        afr = np.zeros((2, FF), np.float32)
        fr0 = np.zeros((FR, FC), np.float32)
        fr0[1:97, 2:194] = aup[0:96]
        fr0[97, 2:194] = aup[96]
        afr[0] = fr0.reshape(FF)
        fr1 = np.zeros((FR, FC), np.float32)
        fr1[1:97, 2:194] = aup[96:192]
        fr1[0, 2:194] = aup[95]
        afr[1] = fr1.reshape(FF)
        af = np.zeros((18, FF), np.float32)
        for h in range(2):
            for t in range(9):
                dlt = _delta(t)
                src = afr[h]
                dst = np.zeros(FF, np.float32)
                if dlt >= 0:
                    dst[:FF - dlt] = src[dlt:]
                else:
                    dst[-dlt:] = src[:FF + dlt]
                af[h * 9 + t] = dst
        maps.append(dict(
            xb=_pad_frame(x[b], BF16),
            x8=_pad_frame(x[b], F8),
            xr=np.ascontiguousarray(xr_all[b].reshape(C, HW)).astype(np.float32),
            w1=w1, w2=w2, w3=w3, kd1=kd[1], kd2=kd[2], cw1=cw1, cw2=cw2,
            g1=g[1], g2=g[2], wa=wa,
            af=np.ascontiguousarray(af).astype(BF16),
            bias=bias))
    return maps


def _diag128(v):
    out = np.zeros((128, 128), np.float32)
    out[np.arange(128), np.arange(128)] = v
    return out


def kernel(**inputs):
    from concourse.bass_utils import run_bass_kernel_spmd

    x = np.asarray(inputs['x'], np.float32)
    d = np.asarray(inputs['d'], np.float32)
    in_maps = _host_precompute(x, d, inputs)

    if 'nc' not in _CACHE:
        _CACHE['nc'] = _build_nc()
    nc = _CACHE['nc']

    try:
        res = run_bass_kernel_spmd(nc, in_maps, list(range(B)))
    except Exception:
        # transient NRT_EXEC_UNIT_UNRECOVERABLE observed on back-to-back
        # runs; a single retry is free and often clears it
        res = run_bass_kernel_spmd(nc, in_maps, list(range(B)))
    out = np.stack([np.asarray(res.results[i]['y'], np.float32).reshape(C, H, W)
                    for i in range(B)])
    return out


# revision 6
# speedup vs baseline: 1.0015x; 1.0015x over previous
"""Trainium2 Bass kernel for the dynamic-attention-block CNN (nn_DAB).

Data-parallel over batch: 8 samples -> 8 NeuronCores. Each core runs the
full per-sample network with activations resident in SBUF as padded
"frames": 128 partitions = 64 channels x 2 image halves, each half a
98x196 zero-padded row-major frame (rows -1..96 / 95..192 of the 192x192
image, cols -2..193).

Conv structure (all single 128-partition matmuls; the two image halves
ride in one instruction via block-diagonal weights):
  - 3x3 convs and dynamic depthwise convs run in fp8e4m3 DoubleRow mode:
    taps are processed in pairs (lhsT [128,2,128], rhs [128,2,N] with the
    pair dim striding between the two tap offsets), 5 passes per conv.
    Weights are pre-scaled by 16 (convs) / 64 (dw) to dodge fp8
    subnormals; the inverse scale is folded into the scalar-engine
    activation that drains PSUM.
  - 1x1 convs + channel-attention gates run in bf16: the x*att residual
    is an extra diagonal-matrix matmul accumulating into the same PSUM
    group, so no vector-engine gating pass exists at all.
  - The additive 32x32-upsampled map is folded into conv2 as one K=18
    matmul pass: 18 partitions hold the 9 tap-shifted copies of the
    upsampled map for each half, weights are the channel-summed conv2
    taps.
Activation outputs are written by the scalar engine directly in the
dtype the consumer needs (fp8 frame for the next conv, bf16 where the
gate needs precision). Residual add + output happens on DVE from f32
x+b3 staged via DMA.
"""

import sys

for _p in ("/opt/trn_rl_repo", "/root/.axon_site/_ro/pypackages"):
    if _p not in sys.path:
        sys.path.insert(0, _p)

import numpy as np
import ml_dtypes

BF16 = ml_dtypes.bfloat16
F8 = ml_dtypes.float8_e4m3

B, C, H, W = 8, 64, 192, 192
HW = H * W
FR, FC = 98, 196          # frame rows / cols per half
FF = FR * FC              # frame elems per partition
Q0 = 1 * FC + 2           # first interior frame position (row 1, col 2)
QL = 96 * FC + 194 - Q0   # sweep length covering all interior rows
TILE = 512
ALPHA = 0.1               # leaky slope
WS = 16.0                 # fp8 conv weight pre-scale
KS = 64.0                 # fp8 dw kernel pre-scale

# bias pack columns
BI_B1, BI_B2, BI_CB1, BI_CB2, BI_Z = range(5)

# DoubleRow tap pairing: (tap_a, tap_b) with taps t = 3*dy + dx,
# delta(t) = (dy-1)*FC + (dx-1).  5 passes cover all 9 taps; the last
# pass's second slot has zero weight (stride 0 keeps the read in-bounds).
PAIRS = [(0, 1), (3, 4), (6, 7), (2, 5), (8, None)]

_CACHE = {}


def _delta(t):
    return (t // 3 - 1) * FC + (t % 3 - 1)


def _qtiles():
    out = []
    q = Q0
    while q < Q0 + QL:
        n = min(TILE, Q0 + QL - q)
        out.append((q, n))
        q += n
    return out


def _build_nc():
    import concourse.bacc as bacc
    import concourse.mybir as mybir
    from concourse import tile

    f32 = mybir.dt.float32
    bf16 = mybir.dt.bfloat16
    f8 = mybir.dt.float8e4
    AF = mybir.ActivationFunctionType
    ALU = mybir.AluOpType
    DR = mybir.MatmulPerfMode.DoubleRow

    nc = bacc.Bacc("TRN2", target_bir_lowering=False, debug=False, num_devices=8)

    xb_d = nc.dram_tensor("xb", [128, FF], bf16, kind="ExternalInput").ap()
    x8_d = nc.dram_tensor("x8", [128, FF], f8, kind="ExternalInput").ap()
    xr_d = nc.dram_tensor("xr", [C, HW], f32, kind="ExternalInput").ap()
    w1_d = nc.dram_tensor("w1", [128, 5, 2, 128], f8, kind="ExternalInput").ap()
    w2_d = nc.dram_tensor("w2", [128, 5, 2, 128], f8, kind="ExternalInput").ap()
    w3_d = nc.dram_tensor("w3", [128, 5, 2, 128], f8, kind="ExternalInput").ap()
    kd1_d = nc.dram_tensor("kd1", [128, 5, 2, 128], f8, kind="ExternalInput").ap()
    kd2_d = nc.dram_tensor("kd2", [128, 5, 2, 128], f8, kind="ExternalInput").ap()
    cw1_d = nc.dram_tensor("cw1", [128, 128], bf16, kind="ExternalInput").ap()
    cw2_d = nc.dram_tensor("cw2", [128, 128], bf16, kind="ExternalInput").ap()
    g1_d = nc.dram_tensor("g1", [128, 128], bf16, kind="ExternalInput").ap()
    g2_d = nc.dram_tensor("g2", [128, 128], bf16, kind="ExternalInput").ap()
    wa_d = nc.dram_tensor("wa", [18, 128], bf16, kind="ExternalInput").ap()
    af_d = nc.dram_tensor("af", [18, FF], bf16, kind="ExternalInput").ap()
    bias_d = nc.dram_tensor("bias", [128, 5], f32, kind="ExternalInput").ap()
    y_d = nc.dram_tensor("y", [C, HW], f32, kind="ExternalOutput").ap()

    qt = _qtiles()

    from contextlib import ExitStack
    with tile.TileContext(nc) as tc, ExitStack() as ctx:
        wpool = ctx.enter_context(tc.tile_pool(name="w", bufs=1))
        fbpool = ctx.enter_context(tc.tile_pool(name="fb", bufs=2))
        f8pool = ctx.enter_context(tc.tile_pool(name="f8", bufs=3))
        stg = ctx.enter_context(tc.tile_pool(name="stg", bufs=2))
        outp = ctx.enter_context(tc.tile_pool(name="outp", bufs=2))
        t1p = ctx.enter_context(tc.tile_pool(name="t1", bufs=4))
        psA = ctx.enter_context(tc.tile_pool(name="psA", bufs=4, space="PSUM"))
        psB = ctx.enter_context(tc.tile_pool(name="psB", bufs=2, space="PSUM"))

        # ---- weights / constants to SBUF ----
        w1 = wpool.tile([128, 5, 2, 128], f8, tag="w1")
        w2 = wpool.tile([128, 5, 2, 128], f8, tag="w2")
        w3 = wpool.tile([128, 5, 2, 128], f8, tag="w3")
        kd1 = wpool.tile([128, 5, 2, 128], f8, tag="kd1")
        kd2 = wpool.tile([128, 5, 2, 128], f8, tag="kd2")
        cw1 = wpool.tile([128, 128], bf16, tag="cw1")
        cw2 = wpool.tile([128, 128], bf16, tag="cw2")
        g1 = wpool.tile([128, 128], bf16, tag="g1")
        g2 = wpool.tile([128, 128], bf16, tag="g2")
        wa = wpool.tile([18, 128], bf16, tag="wa")
        af = wpool.tile([18, FF], bf16, tag="af")
        bias = wpool.tile([128, 5], f32, tag="bias")
        for t, d in ((w1, w1_d), (w2, w2_d), (w3, w3_d), (kd1, kd1_d),
                     (kd2, kd2_d), (cw1, cw1_d), (cw2, cw2_d), (g1, g1_d),
                     (g2, g2_d), (wa, wa_d), (bias, bias_d)):
            nc.gpsimd.dma_start(out=t[...], in_=d)
        nc.scalar.dma_start(out=af[:, :], in_=af_d)

        def cold(col):
            return bias[:, col:col + 1]

        # ---- input frames: host pre-padded, straight contiguous DMA ----
        Xb = fbpool.tile([128, FF], bf16, tag="fb")
        X8 = f8pool.tile([128, FF], f8, tag="f8")
        nchunk = 6
        step = (FF + nchunk - 1) // nchunk
        for k in range(nchunk):
            c0, c1 = k * step, min((k + 1) * step, FF)
            eng = nc.sync if k % 2 == 0 else nc.scalar
            eng.dma_start(out=Xb[:, c0:c1], in_=xb_d[:, c0:c1])
        step8 = (FF + 3) // 4
        for k in range(4):
            c0, c1 = k * step8, min((k + 1) * step8, FF)
            eng = nc.sync if k % 2 == 0 else nc.gpsimd
            eng.dma_start(out=X8[:, c0:c1], in_=x8_d[:, c0:c1])

        def pads_and_halo(m):
            mv = m[:, :].rearrange("p (a b) -> p a b", b=FC)
            nc.gpsimd.memset(mv[0:64, 0, :], 0.0)
            nc.gpsimd.memset(mv[64:128, FR - 1, :], 0.0)
            nc.gpsimd.memset(mv[:, :, 0:2], 0.0)
            nc.gpsimd.memset(mv[:, :, FC - 2:FC], 0.0)
            nc.gpsimd.dma_start(out=mv[0:64, FR - 1, :], in_=mv[64:128, 1, :])
            nc.gpsimd.dma_start(out=mv[64:128, 0, :], in_=mv[0:64, 96, :])

        def dr_rhs(m8, q, n, pair):
            ta, tb = pair
            base = q + _delta(ta)
            stride = 0 if tb is None else _delta(tb) - _delta(ta)
            r = m8[:, base:base + 1].copy()
            r.ap[1] = [stride, 2]
            r.ap.append([1, n])
            return r

        def conv_dr(ps, wsb, m8, q, n):
            for p in range(5):
                nc.tensor.matmul(ps[:, :n], wsb[:, p, :, :], dr_rhs(m8, q, n, PAIRS[p]),
                                 start=(p == 0), stop=(p == 4), perf_mode=DR,
                                 skip_group_check=True)

        def da_stage(inb, in8, kdsb, cwsb, gsb, cb_col, out8):
            # software-pipelined by one tile: PE issues dw(j) before the
            # 1x1+gate of tile j-1 so the in-order PE queue never stalls
            # behind the scalar engine's t1 prelu.
            def tail(prev):
                t1, q, n = prev
                pb = psB.tile([128, TILE], f32, tag="psB")
                nc.tensor.matmul(pb[:, :n], cwsb[:, :], t1[:, :n],
                                 start=True, stop=False, skip_group_check=True)
                nc.tensor.matmul(pb[:, :n], gsb[:, :], inb[:, q:q + n],
                                 start=False, stop=True, skip_group_check=True)
                nc.scalar.activation(out8[:, q:q + n], pb[:, :n], AF.Prelu,
                                     bias=cold(cb_col), alpha=ALPHA)
            prev = None
            for (q, n) in qt:
                pa = psA.tile([128, TILE], f32, tag="psA")
                conv_dr(pa, kdsb, in8, q, n)
                if prev is not None:
                    tail(prev)
                t1 = t1p.tile([128, TILE], bf16, tag="t1")
                nc.scalar.activation(t1[:, :n], pa[:, :n], AF.Prelu,
                                     scale=1.0 / KS, bias=cold(BI_Z), alpha=ALPHA)
                prev = (t1, q, n)
            tail(prev)
            pads_and_halo(out8)

        # ---- network ----
        O1 = f8pool.tile([128, FF], f8, tag="f8")
        da_stage(Xb, X8, kd1, cw1, g1, BI_CB1, O1)

        # conv1 -> prelu -> fp8 frame
        O2 = f8pool.tile([128, FF], f8, tag="f8")
        for (q, n) in qt:
            pa = psA.tile([128, TILE], f32, tag="psA")
            conv_dr(pa, w1, O1, q, n)
            nc.scalar.activation(O2[:, q:q + n], pa[:, :n], AF.Prelu,
                                 scale=1.0 / WS, bias=cold(BI_B1), alpha=ALPHA)
        pads_and_halo(O2)

        # conv2 (+ additive map as a K=18 pass) -> identity+bias -> bf16 + fp8
        O3b = fbpool.tile([128, FF], bf16, tag="fb")
        O38 = f8pool.tile([128, FF], f8, tag="f8")
        for (q, n) in qt:
            pa = psA.tile([128, TILE], f32, tag="psA")
            for p in range(5):
                nc.tensor.matmul(pa[:, :n], w2[:, p, :, :], dr_rhs(O2, q, n, PAIRS[p]),
                                 start=(p == 0), stop=False, perf_mode=DR,
                                 skip_group_check=True)
            nc.tensor.matmul(pa[:, :n], wa[:, :], af[:, q:q + n],
                             start=False, stop=True, skip_group_check=True)
            nc.scalar.activation(O3b[:, q:q + n], pa[:, :n], AF.Identity,
                                 scale=1.0 / WS, bias=cold(BI_B2))
            nc.vector.tensor_copy(O38[:, q:q + n], O3b[:, q:q + n])
        pads_and_halo(O3b)
        pads_and_halo(O38)

        O4 = f8pool.tile([128, FF], f8, tag="f8")
        da_stage(O3b, O38, kd2, cw2, g2, BI_CB2, O4)

        # ---- conv3 fused with +b3 (host-baked into xr) and residual ----
        for k in range(12):
            xs = stg.tile([128, 8, 192], f32, tag="xs")
            nc.sync.dma_start(
                out=xs[0:64, :, :],
                in_=xr_d[:, 8 * k * 192:(8 * k + 8) * 192]
                .rearrange("p (r c) -> p r c", c=192))
            nc.sync.dma_start(
                out=xs[64:128, :, :],
                in_=xr_d[:, (96 + 8 * k) * 192:(96 + 8 * k + 8) * 192]
                .rearrange("p (r c) -> p r c", c=192))
            ot = outp.tile([128, 8, 192], f32, tag="ot")
            for m in range(4):
                r = 1 + 8 * k + 2 * m
                qr = r * FC + 2
                pa = psA.tile([128, TILE], f32, tag="psA")
                conv_dr(pa, w3, O4, qr, 388)
                nc.vector.scalar_tensor_tensor(
                    ot[:, 2 * m:2 * m + 2, :],
                    pa[:, 0:392].rearrange("p (a b) -> p a b", a=2)[:, :, 0:192],
                    1.0 / WS, xs[:, 2 * m:2 * m + 2, :],
                    op0=ALU.mult, op1=ALU.add)
            nc.sync.dma_start(
                out=y_d[:, 8 * k * 192:(8 * k + 8) * 192]
                .rearrange("p (r c) -> p r c", c=192),
                in_=ot[0:64, :, :])
            nc.sync.dma_start(
                out=y_d[:, (96 + 8 * k) * 192:(96 + 8 * k + 8) * 192]
                .rearrange("p (r c) -> p r c", c=192),
                in_=ot[64:128, :, :])

    nc.compile()
    return nc


def _pad_frame(xb, dtype):
    """(64,192,192) fp32 -> (128, FR*FC) dual-half padded frame."""
    fr = np.zeros((128, FR, FC), np.float32)
    fr[0:64, 1:97, 2:194] = xb[:, 0:96, :]
    fr[0:64, 97, 2:194] = xb[:, 96, :]
    fr[64:128, 1:97, 2:194] = xb[:, 96:192, :]
    fr[64:128, 0, 2:194] = xb[:, 95, :]
    return np.ascontiguousarray(fr.reshape(128, FF)).astype(dtype)


def _leaky_np(v):
    return np.where(v >= 0, v, ALPHA * v)


def _host_precompute(x, d, p):
    """Build per-core input maps. p: dict of raw weight arrays."""
    d = d.astype(np.float64)
    kern = {}
    att = {}
    for i in (1, 2):
        kw1, kw2 = p[f'da{i}_kw1'].astype(np.float64), p[f'da{i}_kw2'].astype(np.float64)
        ca1, ca2 = p[f'da{i}_ca1'].astype(np.float64), p[f'da{i}_ca2'].astype(np.float64)
        kern[i] = _leaky_np(d @ kw1.T) @ kw2.T          # (B, 576) [c*9+t]
        z = _leaky_np(d @ ca1.T) @ ca2.T
        att[i] = 1.0 / (1.0 + np.exp(-z))               # (B, 64)
    a32 = _leaky_np(d @ p['add_w1'].astype(np.float64).T) @ \
        p['add_w2'].astype(np.float64).T                # (B, 1024)

    cidx = np.arange(128) % 64
    hidx = np.arange(128) // 64

    def convw_dr(w):
        # (O, C, 3, 3) fp32 -> [128, 5, 2, 128] f8 block-diag DoubleRow taps
        wq = (w.astype(np.float32) * WS).astype(F8).astype(np.float32)
        wt = wq.transpose(1, 2, 3, 0).reshape(64, 9, 64)  # [c, t, o]
        out = np.zeros((128, 5, 2, 128), np.float32)
        for pi, (ta, tb) in enumerate(PAIRS):
            blk = np.zeros((64, 2, 64), np.float32)
            blk[:, 0, :] = wt[:, ta, :]
            if tb is not None:
                blk[:, 1, :] = wt[:, tb, :]
            out[0:64, pi, :, 0:64] = blk
            out[64:128, pi, :, 64:128] = blk
        return np.ascontiguousarray(out).astype(F8)

    def cw_bd(w):
        # (O, C) -> [128, 128] bf16 block-diag: [p, o]
        out = np.zeros((128, 128), np.float32)
        out[0:64, 0:64] = w.T
        out[64:128, 64:128] = w.T
        return np.ascontiguousarray(out).astype(BF16)

    w1 = convw_dr(p['conv1_w'])
    w2 = convw_dr(p['conv2_w'])
    w3 = convw_dr(p['conv3_w'])
    cw1 = cw_bd(p['da1_cw'])
    cw2 = cw_bd(p['da2_cw'])

    # additive-map conv weights: wa[(h,t), o_col] = WS * sum_c conv2_w[o,c,t]
    w2sum = p['conv2_w'].astype(np.float64).sum(axis=1).reshape(64, 9)  # [o, t]
    wa = np.zeros((18, 128), np.float32)
    for h in range(2):
        for t in range(9):
            wa[h * 9 + t, h * 64:(h + 1) * 64] = WS * w2sum[:, t]
    wa = np.ascontiguousarray(wa).astype(BF16)

    xr_all = x.astype(np.float32) + p['conv3_b'].astype(np.float32)[None, :, None, None]

    maps = []
    for b in range(B):
        kd = {}
        for i in (1, 2):
            kc = (kern[i][b].reshape(64, 9).astype(np.float32) * KS) \
                .astype(F8).astype(np.float32)           # [c, t]
            kdl = np.zeros((128, 5, 2, 128), np.float32)
            for pi, (ta, tb) in enumerate(PAIRS):
                kdl[np.arange(128), pi, 0, np.arange(128)] = kc[cidx, ta]
                if tb is not None:
                    kdl[np.arange(128), pi, 1, np.arange(128)] = kc[cidx, tb]
            kd[i] = np.ascontiguousarray(kdl).astype(F8)
        g = {i: np.ascontiguousarray(_diag128(att[i][b][cidx])).astype(BF16)
             for i in (1, 2)}
        bias = np.zeros((128, 5), np.float32)
        bias[:, BI_B1] = p['conv1_b'][cidx]
        bias[:, BI_B2] = p['conv2_b'][cidx]
        bias[:, BI_CB1] = p['da1_cb'][cidx]
        bias[:, BI_CB2] = p['da2_cb'][cidx]

        # additive map frames: 18 partitions = 2 halves x 9 tap shifts
        a = a32[b].astype(np.float32).reshape(32, 32)
        aup = a[np.arange(192) // 6][:, np.arange(192) // 6]  # (192,192)
        afr = np.zeros((2, FF), np.float32)
        fr0 = np.zeros((FR, FC), np.float32)
        fr0[1:97, 2:194] = aup[0:96]
        fr0[97, 2:194] = aup[96]
        afr[0] = fr0.reshape(FF)
        fr1 = np.zeros((FR, FC), np.float32)
        fr1[1:97, 2:194] = aup[96:192]
        fr1[0, 2:194] = aup[95]
        afr[1] = fr1.reshape(FF)
        af = np.zeros((18, FF), np.float32)
        for h in range(2):
            for t in range(9):
                dlt = _delta(t)
                src = afr[h]
                dst = np.zeros(FF, np.float32)
                if dlt >= 0:
                    dst[:FF - dlt] = src[dlt:]
                else:
                    dst[-dlt:] = src[:FF + dlt]
                af[h * 9 + t] = dst
        maps.append(dict(
            xb=_pad_frame(x[b], BF16),
            x8=_pad_frame(x[b], F8),
            xr=np.ascontiguousarray(xr_all[b].reshape(C, HW)).astype(np.float32),
            w1=w1, w2=w2, w3=w3, kd1=kd[1], kd2=kd[2], cw1=cw1, cw2=cw2,
            g1=g[1], g2=g[2], wa=wa,
            af=np.ascontiguousarray(af).astype(BF16),
            bias=bias))
    return maps


def _diag128(v):
    out = np.zeros((128, 128), np.float32)
    out[np.arange(128), np.arange(128)] = v
    return out


def kernel(**inputs):
    from concourse.bass_utils import run_bass_kernel_spmd

    x = np.asarray(inputs['x'], np.float32)
    d = np.asarray(inputs['d'], np.float32)
    in_maps = _host_precompute(x, d, inputs)

    if 'nc' not in _CACHE:
        _CACHE['nc'] = _build_nc()
    nc = _CACHE['nc']

    try:
        res = run_bass_kernel_spmd(nc, in_maps, list(range(B)))
    except Exception:
        # transient NRT_EXEC_UNIT_UNRECOVERABLE observed on back-to-back
        # runs; a single retry is free and often clears it
        res = run_bass_kernel_spmd(nc, in_maps, list(range(B)))
    out = np.stack([np.asarray(res.results[i]['y'], np.float32).reshape(C, H, W)
                    for i in range(B)])
    return out


# revision 12
# speedup vs baseline: 1.0716x; 1.0700x over previous
"""Trainium2 Bass kernel for the dynamic-attention-block CNN (nn_DAB).

Data-parallel over batch: 8 samples -> 8 NeuronCores. Each core runs the
full per-sample network with activations resident in SBUF as padded
"frames": 128 partitions = 64 channels x 2 image halves, each half a
98x196 zero-padded row-major frame (rows -1..96 / 95..192 of the 192x192
image, cols -2..193).

Conv structure (all single 128-partition matmuls; the two image halves
ride in one instruction via block-diagonal weights):
  - 3x3 convs and dynamic depthwise convs run in fp8e4m3 DoubleRow mode:
    taps are processed in pairs (lhsT [128,2,128], rhs [128,2,N] with the
    pair dim striding between the two tap offsets), 5 passes per conv.
    Weights are pre-scaled by 16 (convs) / 64 (dw) to dodge fp8
    subnormals; the inverse scale is folded into the scalar-engine
    activation that drains PSUM.
  - 1x1 convs + channel-attention gates run in bf16: the x*att residual
    is an extra diagonal-matrix matmul accumulating into the same PSUM
    group, so no vector-engine gating pass exists at all.
  - The additive 32x32-upsampled map is folded into conv2 as one K=18
    matmul pass: 18 partitions hold the 9 tap-shifted copies of the
    upsampled map for each half, weights are the channel-summed conv2
    taps.
Activation outputs are written by the scalar engine directly in the
dtype the consumer needs (fp8 frame for the next conv, bf16 where the
gate needs precision). Residual add + output happens on DVE from f32
x+b3 staged via DMA.
"""

import sys

for _p in ("/opt/trn_rl_repo", "/root/.axon_site/_ro/pypackages"):
    if _p not in sys.path:
        sys.path.insert(0, _p)

import numpy as np
import ml_dtypes

BF16 = ml_dtypes.bfloat16
F8 = ml_dtypes.float8_e4m3

B, C, H, W = 8, 64, 192, 192
HW = H * W
FR, FC = 98, 196          # frame rows / cols per half
FF = FR * FC              # frame elems per partition
Q0 = 1 * FC + 2           # first interior frame position (row 1, col 2)
QL = 96 * FC + 194 - Q0   # sweep length covering all interior rows
TILE = 512
ALPHA = 0.1               # leaky slope
WS = 16.0                 # fp8 conv weight pre-scale
KS = 64.0                 # fp8 dw kernel pre-scale

# bias pack columns
BI_B1, BI_B2, BI_CB1, BI_CB2, BI_Z = range(5)

# DoubleRow tap pairing: (tap_a, tap_b) with taps t = 3*dy + dx,
# delta(t) = (dy-1)*FC + (dx-1).  5 passes cover all 9 taps; the last
# pass's second slot has zero weight (stride 0 keeps the read in-bounds).
PAIRS = [(0, 1), (3, 4), (6, 7), (2, 5), (8, None)]

_CACHE = {}


def _delta(t):
    return (t // 3 - 1) * FC + (t % 3 - 1)


def _qtiles():
    out = []
    q = Q0
    while q < Q0 + QL:
        n = min(TILE, Q0 + QL - q)
        out.append((q, n))
        q += n
    return out


def _build_nc():
    import concourse.bacc as bacc
    import concourse.mybir as mybir
    from concourse import tile

    f32 = mybir.dt.float32
    bf16 = mybir.dt.bfloat16
    f8 = mybir.dt.float8e4
    AF = mybir.ActivationFunctionType
    ALU = mybir.AluOpType
    DR = mybir.MatmulPerfMode.DoubleRow

    nc = bacc.Bacc("TRN2", target_bir_lowering=False, debug=False, num_devices=8)

    xb_d = nc.dram_tensor("xb", [128, FF], bf16, kind="ExternalInput").ap()
    x8_d = nc.dram_tensor("x8", [128, FF], f8, kind="ExternalInput").ap()
    xr_d = nc.dram_tensor("xr", [C, HW], f32, kind="ExternalInput").ap()
    w1_d = nc.dram_tensor("w1", [128, 5, 2, 128], f8, kind="ExternalInput").ap()
    w2_d = nc.dram_tensor("w2", [128, 5, 2, 128], f8, kind="ExternalInput").ap()
    w3_d = nc.dram_tensor("w3", [128, 5, 2, 128], f8, kind="ExternalInput").ap()
    kd1_d = nc.dram_tensor("kd1", [128, 5, 2, 128], f8, kind="ExternalInput").ap()
    kd2_d = nc.dram_tensor("kd2", [128, 5, 2, 128], f8, kind="ExternalInput").ap()
    cw1_d = nc.dram_tensor("cw1", [128, 128], bf16, kind="ExternalInput").ap()
    cw2_d = nc.dram_tensor("cw2", [128, 128], bf16, kind="ExternalInput").ap()
    g1_d = nc.dram_tensor("g1", [128, 128], bf16, kind="ExternalInput").ap()
    g2_d = nc.dram_tensor("g2", [128, 128], bf16, kind="ExternalInput").ap()
    wa_d = nc.dram_tensor("wa", [18, 128], bf16, kind="ExternalInput").ap()
    af_d = nc.dram_tensor("af", [18, FF], bf16, kind="ExternalInput").ap()
    bias_d = nc.dram_tensor("bias", [128, 5], f32, kind="ExternalInput").ap()
    y_d = nc.dram_tensor("y", [C, HW], f32, kind="ExternalOutput").ap()

    qt = _qtiles()

    from contextlib import ExitStack
    with tile.TileContext(nc) as tc, ExitStack() as ctx:
        wpool = ctx.enter_context(tc.tile_pool(name="w", bufs=1))
        fbpool = ctx.enter_context(tc.tile_pool(name="fb", bufs=2))
        f8pool = ctx.enter_context(tc.tile_pool(name="f8", bufs=3))
        stg = ctx.enter_context(tc.tile_pool(name="stg", bufs=3))
        outp = ctx.enter_context(tc.tile_pool(name="outp", bufs=3))
        t1p = ctx.enter_context(tc.tile_pool(name="t1", bufs=3))
        psA = ctx.enter_context(tc.tile_pool(name="psA", bufs=4, space="PSUM"))
        psB = ctx.enter_context(tc.tile_pool(name="psB", bufs=4, space="PSUM"))

        # ---- weights / constants to SBUF ----
        w1 = wpool.tile([128, 5, 2, 128], f8, tag="w1")
        w2 = wpool.tile([128, 5, 2, 128], f8, tag="w2")
        w3 = wpool.tile([128, 5, 2, 128], f8, tag="w3")
        kd1 = wpool.tile([128, 5, 2, 128], f8, tag="kd1")
        kd2 = wpool.tile([128, 5, 2, 128], f8, tag="kd2")
        cw1 = wpool.tile([128, 128], bf16, tag="cw1")
        cw2 = wpool.tile([128, 128], bf16, tag="cw2")
        g1 = wpool.tile([128, 128], bf16, tag="g1")
        g2 = wpool.tile([128, 128], bf16, tag="g2")
        wa = wpool.tile([18, 128], bf16, tag="wa")
        af = wpool.tile([18, FF], bf16, tag="af")
        bias = wpool.tile([128, 5], f32, tag="bias")
        for t, d in ((w1, w1_d), (w2, w2_d), (w3, w3_d), (kd1, kd1_d),
                     (kd2, kd2_d), (cw1, cw1_d), (cw2, cw2_d), (g1, g1_d),
                     (g2, g2_d), (wa, wa_d), (bias, bias_d)):
            nc.gpsimd.dma_start(out=t[...], in_=d)
        nc.scalar.dma_start(out=af[:, :], in_=af_d)

        def cold(col):
            return bias[:, col:col + 1]

        # ---- PE warmup: ~3us of throwaway matmuls so the p-state ramp
        # finishes while the input DMAs stream in ----
        wrm = wpool.tile([128, TILE], bf16, tag="wrm")
        nc.gpsimd.memset(wrm[:, :], 0.0)
        pw = psA.tile([128, TILE], f32, tag="psA")
        for _ in range(16):
            nc.tensor.matmul(pw[:, :], wrm[:, 0:128], wrm[:, :],
                             start=True, stop=True, skip_group_check=True)

        # ---- input frames: host pre-padded, contiguous DMA. The fp8
        # frame (needed first, by dw1) goes first on all three queues ----
        Xb = fbpool.tile([128, FF], bf16, tag="fb")
        X8 = f8pool.tile([128, FF], f8, tag="f8")
        qs = (nc.sync, nc.scalar, nc.gpsimd)
        step8 = (FF + 2) // 3
        for k in range(3):
            c0, c1 = k * step8, min((k + 1) * step8, FF)
            qs[k].dma_start(out=X8[:, c0:c1], in_=x8_d[:, c0:c1])
        nchunk = 6
        step = (FF + nchunk - 1) // nchunk
        for k in range(nchunk):
            c0, c1 = k * step, min((k + 1) * step, FF)
            qs[k % 3].dma_start(out=Xb[:, c0:c1], in_=xb_d[:, c0:c1])

        def pads_and_halo(m):
            mv = m[:, :].rearrange("p (a b) -> p a b", b=FC)
            nc.gpsimd.memset(mv[0:64, 0, :], 0.0)
            nc.gpsimd.memset(mv[64:128, FR - 1, :], 0.0)
            nc.gpsimd.memset(mv[:, :, 0:2], 0.0)
            nc.gpsimd.memset(mv[:, :, FC - 2:FC], 0.0)
            nc.gpsimd.dma_start(out=mv[0:64, FR - 1, :], in_=mv[64:128, 1, :])
            nc.gpsimd.dma_start(out=mv[64:128, 0, :], in_=mv[0:64, 96, :])

        def dr_rhs(m8, q, n, pair):
            ta, tb = pair
            base = q + _delta(ta)
            stride = 0 if tb is None else _delta(tb) - _delta(ta)
            r = m8[:, base:base + 1].copy()
            r.ap[1] = [stride, 2]
            r.ap.append([1, n])
            return r

        def conv_dr(ps, wsb, m8, q, n):
            for p in range(5):
                nc.tensor.matmul(ps[:, :n], wsb[:, p, :, :], dr_rhs(m8, q, n, PAIRS[p]),
                                 start=(p == 0), stop=(p == 4), perf_mode=DR,
                                 skip_group_check=True)

        def da_stage(inb, in8, kdsb, cwsb, gsb, cb_col, out8):
            # software-pipelined by one tile: PE issues dw(j) before the
            # 1x1+gate of tile j-1 so the in-order PE queue never stalls
            # behind the scalar engine's t1 prelu.
            def tail(prev):
                t1, q, n = prev
                pb = psB.tile([128, TILE], f32, tag="psB")
                nc.tensor.matmul(pb[:, :n], cwsb[:, :], t1[:, :n],
                                 start=True, stop=False, skip_group_check=True)
                nc.tensor.matmul(pb[:, :n], gsb[:, :], inb[:, q:q + n],
                                 start=False, stop=True, skip_group_check=True)
                nc.scalar.activation(out8[:, q:q + n], pb[:, :n], AF.Prelu,
                                     bias=cold(cb_col), alpha=ALPHA)
            prev = None
            for (q, n) in qt:
                pa = psA.tile([128, TILE], f32, tag="psA")
                conv_dr(pa, kdsb, in8, q, n)
                if prev is not None:
                    tail(prev)
                t1 = t1p.tile([128, TILE], bf16, tag="t1")
                nc.scalar.activation(t1[:, :n], pa[:, :n], AF.Prelu,
                                     scale=1.0 / KS, bias=cold(BI_Z), alpha=ALPHA)
                prev = (t1, q, n)
            tail(prev)
            pads_and_halo(out8)

        # ---- network ----
        O1 = f8pool.tile([128, FF], f8, tag="f8")
        da_stage(Xb, X8, kd1, cw1, g1, BI_CB1, O1)

        # conv1 -> prelu -> fp8 frame
        O2 = f8pool.tile([128, FF], f8, tag="f8")
        for (q, n) in qt:
            pa = psA.tile([128, TILE], f32, tag="psA")
            conv_dr(pa, w1, O1, q, n)
            nc.scalar.activation(O2[:, q:q + n], pa[:, :n], AF.Prelu,
                                 scale=1.0 / WS, bias=cold(BI_B1), alpha=ALPHA)
        pads_and_halo(O2)

        # conv2 (+ additive map as a K=18 pass) -> identity+bias -> bf16 + fp8
        O3b = fbpool.tile([128, FF], bf16, tag="fb")
        O38 = f8pool.tile([128, FF], f8, tag="f8")
        for (q, n) in qt:
            pa = psA.tile([128, TILE], f32, tag="psA")
            for p in range(5):
                nc.tensor.matmul(pa[:, :n], w2[:, p, :, :], dr_rhs(O2, q, n, PAIRS[p]),
                                 start=(p == 0), stop=False, perf_mode=DR,
                                 skip_group_check=True)
            nc.tensor.matmul(pa[:, :n], wa[:, :], af[:, q:q + n],
                             start=False, stop=True, skip_group_check=True)
            nc.scalar.activation(O3b[:, q:q + n], pa[:, :n], AF.Identity,
                                 scale=1.0 / WS, bias=cold(BI_B2))
            nc.vector.tensor_copy(O38[:, q:q + n], O3b[:, q:q + n])
        pads_and_halo(O3b)
        pads_and_halo(O38)

        O4 = f8pool.tile([128, FF], f8, tag="f8")
        da_stage(O3b, O38, kd2, cw2, g2, BI_CB2, O4)

        # ---- conv3 fused with +b3 (host-baked into xr) and residual.
        # xs prefetched 2 chunks ahead; DMAs fanned across all 3 queues ----
        NCH = 16
        RCH = 6
        xs_tiles = {}

        def fetch_xs(k):
            xs = stg.tile([128, RCH, 192], f32, tag="xs")
            qs[(2 * k) % 3].dma_start(
                out=xs[0:64, :, :],
                in_=xr_d[:, RCH * k * 192:(RCH * k + RCH) * 192]
                .rearrange("p (r c) -> p r c", c=192))
            qs[(2 * k + 1) % 3].dma_start(
                out=xs[64:128, :, :],
                in_=xr_d[:, (96 + RCH * k) * 192:(96 + RCH * k + RCH) * 192]
                .rearrange("p (r c) -> p r c", c=192))
            xs_tiles[k] = xs

        fetch_xs(0)
        fetch_xs(1)
        for k in range(NCH):
            xs = xs_tiles.pop(k)
            ot = outp.tile([128, RCH, 192], f32, tag="ot")
            for m in range(RCH // 2):
                r = 1 + RCH * k + 2 * m
                qr = r * FC + 2
                pa = psA.tile([128, TILE], f32, tag="psA")
                conv_dr(pa, w3, O4, qr, 388)
                if m == 0 and k + 2 < NCH:
                    fetch_xs(k + 2)
                nc.vector.scalar_tensor_tensor(
                    ot[:, 2 * m:2 * m + 2, :],
                    pa[:, 0:392].rearrange("p (a b) -> p a b", a=2)[:, :, 0:192],
                    1.0 / WS, xs[:, 2 * m:2 * m + 2, :],
                    op0=ALU.mult, op1=ALU.add)
            qs[(2 * k) % 3].dma_start(
                out=y_d[:, RCH * k * 192:(RCH * k + RCH) * 192]
                .rearrange("p (r c) -> p r c", c=192),
                in_=ot[0:64, :, :])
            qs[(2 * k + 1) % 3].dma_start(
                out=y_d[:, (96 + RCH * k) * 192:(96 + RCH * k + RCH) * 192]
                .rearrange("p (r c) -> p r c", c=192),
                in_=ot[64:128, :, :])

    nc.compile()
    return nc


def _pad_frame(xb, dtype):
    """(64,192,192) fp32 -> (128, FR*FC) dual-half padded frame."""
    fr = np.zeros((128, FR, FC), np.float32)
    fr[0:64, 1:97, 2:194] = xb[:, 0:96, :]
    fr[0:64, 97, 2:194] = xb[:, 96, :]
    fr[64:128, 1:97, 2:194] = xb[:, 96:192, :]
    fr[64:128, 0, 2:194] = xb[:, 95, :]
    return np.ascontiguousarray(fr.reshape(128, FF)).astype(dtype)


def _leaky_np(v):
    return np.where(v >= 0, v, ALPHA * v)


def _host_precompute(x, d, p):
    """Build per-core input maps. p: dict of raw weight arrays."""
    d = d.astype(np.float64)
    kern = {}
    att = {}
    for i in (1, 2):
        kw1, kw2 = p[f'da{i}_kw1'].astype(np.float64), p[f'da{i}_kw2'].astype(np.float64)
        ca1, ca2 = p[f'da{i}_ca1'].astype(np.float64), p[f'da{i}_ca2'].astype(np.float64)
        kern[i] = _leaky_np(d @ kw1.T) @ kw2.T          # (B, 576) [c*9+t]
        z = _leaky_np(d @ ca1.T) @ ca2.T
        att[i] = 1.0 / (1.0 + np.exp(-z))               # (B, 64)
    a32 = _leaky_np(d @ p['add_w1'].astype(np.float64).T) @ \
        p['add_w2'].astype(np.float64).T                # (B, 1024)

    cidx = np.arange(128) % 64
    hidx = np.arange(128) // 64

    def convw_dr(w):
        # (O, C, 3, 3) fp32 -> [128, 5, 2, 128] f8 block-diag DoubleRow taps
        wq = (w.astype(np.float32) * WS).astype(F8).astype(np.float32)
        wt = wq.transpose(1, 2, 3, 0).reshape(64, 9, 64)  # [c, t, o]
        out = np.zeros((128, 5, 2, 128), np.float32)
        for pi, (ta, tb) in enumerate(PAIRS):
            blk = np.zeros((64, 2, 64), np.float32)
            blk[:, 0, :] = wt[:, ta, :]
            if tb is not None:
                blk[:, 1, :] = wt[:, tb, :]
            out[0:64, pi, :, 0:64] = blk
            out[64:128, pi, :, 64:128] = blk
        return np.ascontiguousarray(out).astype(F8)

    def cw_bd(w):
        # (O, C) -> [128, 128] bf16 block-diag: [p, o]
        out = np.zeros((128, 128), np.float32)
        out[0:64, 0:64] = w.T
        out[64:128, 64:128] = w.T
        return np.ascontiguousarray(out).astype(BF16)

    w1 = convw_dr(p['conv1_w'])
    w2 = convw_dr(p['conv2_w'])
    w3 = convw_dr(p['conv3_w'])
    cw1 = cw_bd(p['da1_cw'])
    cw2 = cw_bd(p['da2_cw'])

    # additive-map conv weights: wa[(h,t), o_col] = WS * sum_c conv2_w[o,c,t]
    w2sum = p['conv2_w'].astype(np.float64).sum(axis=1).reshape(64, 9)  # [o, t]
    wa = np.zeros((18, 128), np.float32)
    for h in range(2):
        for t in range(9):
            wa[h * 9 + t, h * 64:(h + 1) * 64] = WS * w2sum[:, t]
    wa = np.ascontiguousarray(wa).astype(BF16)

    xr_all = x.astype(np.float32) + p['conv3_b'].astype(np.float32)[None, :, None, None]

    maps = []
    for b in range(B):
        kd = {}
        for i in (1, 2):
            kc = (kern[i][b].reshape(64, 9).astype(np.float32) * KS) \
                .astype(F8).astype(np.float32)           # [c, t]
            kdl = np.zeros((128, 5, 2, 128), np.float32)
            for pi, (ta, tb) in enumerate(PAIRS):
                kdl[np.arange(128), pi, 0, np.arange(128)] = kc[cidx, ta]
                if tb is not None:
                    kdl[np.arange(128), pi, 1, np.arange(128)] = kc[cidx, tb]
            kd[i] = np.ascontiguousarray(kdl).astype(F8)
        g = {i: np.ascontiguousarray(_diag128(att[i][b][cidx])).astype(BF16)
             for i in (1, 2)}
        bias = np.zeros((128, 5), np.float32)
        bias[:, BI_B1] = p['conv1_b'][cidx]
        bias[:, BI_B2] = p['conv2_b'][cidx]
        bias[:, BI_CB1] = p['da1_cb'][cidx]
        bias[:, BI_CB2] = p['da2_cb'][cidx]

        # additive map frames: 18 partitions = 2 halves x 9 tap shifts
        a = a32[b].astype(np.float32).reshape(32, 32)
        aup = a[np.arange(192) // 6][:, np.arange(192) // 6]  # (192,192)
        afr = np.zeros((2, FF), np.float32)
        fr0 = np.zeros((FR, FC), np.float32)
        fr0[1:97, 2:194] = aup[0:96]
        fr0[97, 2:194] = aup[96]
        afr[0] = fr0.reshape(FF)
        fr1 = np.zeros((FR, FC), np.float32)
        fr1[1:97, 2:194] = aup[96:192]
        fr1[0, 2:194] = aup[95]
        afr[1] = fr1.reshape(FF)
        af = np.zeros((18, FF), np.float32)
        for h in range(2):
            for t in range(9):
                dlt = _delta(t)
                src = afr[h]
                dst = np.zeros(FF, np.float32)
                if dlt >= 0:
                    dst[:FF - dlt] = src[dlt:]
                else:
                    dst[-dlt:] = src[:FF + dlt]
                af[h * 9 + t] = dst
        maps.append(dict(
            xb=_pad_frame(x[b], BF16),
            x8=_pad_frame(x[b], F8),
            xr=np.ascontiguousarray(xr_all[b].reshape(C, HW)).astype(np.float32),
            w1=w1, w2=w2, w3=w3, kd1=kd[1], kd2=kd[2], cw1=cw1, cw2=cw2,
            g1=g[1], g2=g[2], wa=wa,
            af=np.ascontiguousarray(af).astype(BF16),
            bias=bias))
    return maps


def _diag128(v):
    out = np.zeros((128, 128), np.float32)
    out[np.arange(128), np.arange(128)] = v
    return out


def kernel(**inputs):
    from concourse.bass_utils import run_bass_kernel_spmd

    x = np.asarray(inputs['x'], np.float32)
    d = np.asarray(inputs['d'], np.float32)
    in_maps = _host_precompute(x, d, inputs)

    if 'nc' not in _CACHE:
        _CACHE['nc'] = _build_nc()
    nc = _CACHE['nc']

    try:
        res = run_bass_kernel_spmd(nc, in_maps, list(range(B)))
    except Exception:
        # transient NRT_EXEC_UNIT_UNRECOVERABLE observed on back-to-back
        # runs; a single retry is free and often clears it
        res = run_bass_kernel_spmd(nc, in_maps, list(range(B)))
    out = np.stack([np.asarray(res.results[i]['y'], np.float32).reshape(C, H, W)
                    for i in range(B)])
    return out


# revision 14
# speedup vs baseline: 1.0750x; 1.0031x over previous
"""Trainium2 Bass kernel for the dynamic-attention-block CNN (nn_DAB).

Data-parallel over batch: 8 samples -> 8 NeuronCores. Each core runs the
full per-sample network with activations resident in SBUF as padded
"frames": 128 partitions = 64 channels x 2 image halves, each half a
98x196 zero-padded row-major frame (rows -1..96 / 95..192 of the 192x192
image, cols -2..193).

Conv structure (all single 128-partition matmuls; the two image halves
ride in one instruction via block-diagonal weights):
  - 3x3 convs and dynamic depthwise convs run in fp8e4m3 DoubleRow mode:
    taps are processed in pairs (lhsT [128,2,128], rhs [128,2,N] with the
    pair dim striding between the two tap offsets), 5 passes per conv.
    Weights are pre-scaled by 16 (convs) / 64 (dw) to dodge fp8
    subnormals; the inverse scale is folded into the scalar-engine
    activation that drains PSUM.
  - 1x1 convs + channel-attention gates run in bf16: the x*att residual
    is an extra diagonal-matrix matmul accumulating into the same PSUM
    group, so no vector-engine gating pass exists at all.
  - The additive 32x32-upsampled map is folded into conv2 as one K=18
    matmul pass: 18 partitions hold the 9 tap-shifted copies of the
    upsampled map for each half, weights are the channel-summed conv2
    taps.
Activation outputs are written by the scalar engine directly in the
dtype the consumer needs (fp8 frame for the next conv, bf16 where the
gate needs precision). Residual add + output happens on DVE from f32
x+b3 staged via DMA.
"""

import sys

for _p in ("/opt/trn_rl_repo", "/root/.axon_site/_ro/pypackages"):
    if _p not in sys.path:
        sys.path.insert(0, _p)

import numpy as np
import ml_dtypes

BF16 = ml_dtypes.bfloat16
F8 = ml_dtypes.float8_e4m3

B, C, H, W = 8, 64, 192, 192
HW = H * W
FR, FC = 98, 196          # frame rows / cols per half
FF = FR * FC              # frame elems per partition
Q0 = 1 * FC + 2           # first interior frame position (row 1, col 2)
QL = 96 * FC + 194 - Q0   # sweep length covering all interior rows
TILE = 512
ALPHA = 0.1               # leaky slope
WS = 16.0                 # fp8 conv weight pre-scale
KS = 64.0                 # fp8 dw kernel pre-scale

# bias pack columns
BI_B1, BI_B2, BI_CB1, BI_CB2, BI_Z = range(5)

# DoubleRow tap pairing: (tap_a, tap_b) with taps t = 3*dy + dx,
# delta(t) = (dy-1)*FC + (dx-1).  5 passes cover all 9 taps; the last
# pass's second slot has zero weight (stride 0 keeps the read in-bounds).
PAIRS = [(0, 1), (3, 4), (6, 7), (2, 5), (8, None)]

_CACHE = {}


def _delta(t):
    return (t // 3 - 1) * FC + (t % 3 - 1)


def _qtiles():
    out = []
    q = Q0
    while q < Q0 + QL:
        n = min(TILE, Q0 + QL - q)
        out.append((q, n))
        q += n
    return out


def _build_nc():
    import concourse.bacc as bacc
    import concourse.mybir as mybir
    from concourse import tile

    f32 = mybir.dt.float32
    bf16 = mybir.dt.bfloat16
    f8 = mybir.dt.float8e4
    AF = mybir.ActivationFunctionType
    ALU = mybir.AluOpType
    DR = mybir.MatmulPerfMode.DoubleRow

    nc = bacc.Bacc("TRN2", target_bir_lowering=False, debug=False, num_devices=8)

    xb_d = nc.dram_tensor("xb", [128, FF], bf16, kind="ExternalInput").ap()
    x8_d = nc.dram_tensor("x8", [128, FF], f8, kind="ExternalInput").ap()
    rid_d = nc.dram_tensor("rid", [128, 128], bf16, kind="ExternalInput").ap()
    b3r_d = nc.dram_tensor("b3r", [1, 128], bf16, kind="ExternalInput").ap()
    w1_d = nc.dram_tensor("w1", [128, 5, 2, 128], f8, kind="ExternalInput").ap()
    w2_d = nc.dram_tensor("w2", [128, 5, 2, 128], f8, kind="ExternalInput").ap()
    w3_d = nc.dram_tensor("w3", [128, 5, 2, 128], f8, kind="ExternalInput").ap()
    kd1_d = nc.dram_tensor("kd1", [128, 5, 2, 128], f8, kind="ExternalInput").ap()
    kd2_d = nc.dram_tensor("kd2", [128, 5, 2, 128], f8, kind="ExternalInput").ap()
    cw1_d = nc.dram_tensor("cw1", [128, 128], bf16, kind="ExternalInput").ap()
    cw2_d = nc.dram_tensor("cw2", [128, 128], bf16, kind="ExternalInput").ap()
    g1_d = nc.dram_tensor("g1", [128, 128], bf16, kind="ExternalInput").ap()
    g2_d = nc.dram_tensor("g2", [128, 128], bf16, kind="ExternalInput").ap()
    wa_d = nc.dram_tensor("wa", [18, 128], bf16, kind="ExternalInput").ap()
    af_d = nc.dram_tensor("af", [18, FF], bf16, kind="ExternalInput").ap()
    bias_d = nc.dram_tensor("bias", [128, 5], f32, kind="ExternalInput").ap()
    y_d = nc.dram_tensor("y", [C, HW], f32, kind="ExternalOutput").ap()

    # row-pair sweep tiles: 48 tiles of 2 image rows (392 frame cols)
    qt = [((2 * i + 1) * FC + 2, 2 * FC) for i in range(48)]

    from contextlib import ExitStack
    with tile.TileContext(nc) as tc, ExitStack() as ctx:
        wpool = ctx.enter_context(tc.tile_pool(name="w", bufs=1))
        fbpool = ctx.enter_context(tc.tile_pool(name="fb", bufs=2))
        f8pool = ctx.enter_context(tc.tile_pool(name="f8", bufs=3))
        t1p = ctx.enter_context(tc.tile_pool(name="t1", bufs=4))
        outp = ctx.enter_context(tc.tile_pool(name="outp", bufs=4))
        psA = ctx.enter_context(tc.tile_pool(name="psA", bufs=4, space="PSUM"))
        psB = ctx.enter_context(tc.tile_pool(name="psB", bufs=4, space="PSUM"))

        # ---- weights / constants to SBUF ----
        w1 = wpool.tile([128, 5, 2, 128], f8, tag="w1")
        w2 = wpool.tile([128, 5, 2, 128], f8, tag="w2")
        w3 = wpool.tile([128, 5, 2, 128], f8, tag="w3")
        kd1 = wpool.tile([128, 5, 2, 128], f8, tag="kd1")
        kd2 = wpool.tile([128, 5, 2, 128], f8, tag="kd2")
        cw1 = wpool.tile([128, 128], bf16, tag="cw1")
        cw2 = wpool.tile([128, 128], bf16, tag="cw2")
        g1 = wpool.tile([128, 128], bf16, tag="g1")
        g2 = wpool.tile([128, 128], bf16, tag="g2")
        rid = wpool.tile([128, 128], bf16, tag="rid")     # diag(WS) residual
        ones = wpool.tile([1, 2 * FC], bf16, tag="ones")  # const-1 row
        b3r = wpool.tile([1, 128], bf16, tag="b3r")       # WS*b3 row
        wa = wpool.tile([18, 128], bf16, tag="wa")
        af = wpool.tile([18, FF], bf16, tag="af")
        bias = wpool.tile([128, 5], f32, tag="bias")
        for t, d in ((w1, w1_d), (w2, w2_d), (w3, w3_d), (kd1, kd1_d),
                     (kd2, kd2_d), (cw1, cw1_d), (cw2, cw2_d), (g1, g1_d),
                     (g2, g2_d), (rid, rid_d), (b3r, b3r_d), (wa, wa_d),
                     (bias, bias_d)):
            nc.gpsimd.dma_start(out=t[...], in_=d)
        nc.scalar.dma_start(out=af[:, :], in_=af_d)
        nc.vector.memset(ones[:, :], 1.0)

        def cold(col):
            return bias[:, col:col + 1]

        # ---- PE warmup: ~4us of throwaway matmuls so the p-state ramp
        # finishes while the input DMAs stream in ----
        wrm = wpool.tile([128, TILE], bf16, tag="wrm")
        nc.vector.memset(wrm[:, :], 0.0)
        pw = psA.tile([128, TILE], f32, tag="psA")
        for _ in range(16):
            nc.tensor.matmul(pw[:, :], wrm[:, 0:128], wrm[:, :],
                             start=True, stop=True, skip_group_check=True)

        # ---- input frames (host pre-padded); fp8 frame first ----
        Xb = fbpool.tile([128, FF], bf16, tag="fb")
        X8 = f8pool.tile([128, FF], f8, tag="f8")
        O1 = f8pool.tile([128, FF], f8, tag="f8")
        O2 = f8pool.tile([128, FF], f8, tag="f8")
        O3b = fbpool.tile([128, FF], bf16, tag="fb")
        qs = (nc.sync, nc.scalar, nc.gpsimd)
        step8 = (FF + 2) // 3
        for k in range(3):
            c0, c1 = k * step8, min((k + 1) * step8, FF)
            qs[k].dma_start(out=X8[:, c0:c1], in_=x8_d[:, c0:c1])
        nchunk = 6
        step = (FF + nchunk - 1) // nchunk
        for k in range(nchunk):
            c0, c1 = k * step, min((k + 1) * step, FF)
            qs[k % 3].dma_start(out=Xb[:, c0:c1], in_=xb_d[:, c0:c1])

        def v3(m):
            return m[:, :].rearrange("p (a b) -> p a b", b=FC)

        # one-time pad zeroing for frame buffers not filled by host DMA.
        # Interior writes never touch pads again, so pads stay zero across
        # all later reuses of these pool buffers.
        for m in (O1, O2, O3b):
            mv = v3(m)
            nc.gpsimd.memset(mv[0:64, 0, :], 0.0)
            nc.gpsimd.memset(mv[64:128, FR - 1, :], 0.0)
            nc.gpsimd.memset(mv[:, :, 0:2], 0.0)
            nc.gpsimd.memset(mv[:, :, FC - 2:FC], 0.0)

        def halo(m):
            mv = v3(m)
            nc.gpsimd.dma_start(out=mv[0:64, FR - 1, :], in_=mv[64:128, 1, :])
            nc.gpsimd.dma_start(out=mv[64:128, 0, :], in_=mv[0:64, 96, :])

        def dr_rhs(m8, q, n, pair):
            ta, tb = pair
            base = q + _delta(ta)
            stride = 0 if tb is None else _delta(tb) - _delta(ta)
            n = min(n, FF - base - max(stride, 0))
            r = m8[:, base:base + 1].copy()
            r.ap[1] = [stride, 2]
            r.ap.append([1, n])
            return r, n

        def conv_dr(ps, wsb, m8, q, n):
            # P0 (top-left taps) never clamps, so it is the start pass and
            # always covers the full tile; clamped later passes only lose
            # tail columns that are pad positions, never emitted.
            for p in range(5):
                rhs, np_ = dr_rhs(m8, q, n, PAIRS[p])
                nc.tensor.matmul(ps[:, :np_], wsb[:, p, :, :], rhs,
                                 start=(p == 0), stop=(p == 4), perf_mode=DR,
                                 skip_group_check=True)

        def act_out(dst, src_ps, q, n, func, **kw):
            # interior-only write: rows of the pair, cols 2:194
            r = q // FC
            dv = v3(dst)[:, r:r + 2, 2:194]
            sv = src_ps[:, :n].rearrange("p (a b) -> p a b", b=FC)[:, :, 0:192]
            nc.scalar.activation(dv, sv, func, **kw)

        def da_stage(inb, in8, kdsb, cwsb, gsb, cb_col, out8):
            # software-pipelined by one tile: PE issues dw(j) before the
            # 1x1+gate of tile j-1 so the in-order PE queue never stalls
            # behind the scalar engine's t1 prelu.
            def tail(prev):
                t1, q, n = prev
                pb = psB.tile([128, 2 * FC], f32, tag="psB")
                nc.tensor.matmul(pb[:, :n], cwsb[:, :], t1[:, :n],
                                 start=True, stop=False, skip_group_check=True)
                nc.tensor.matmul(pb[:, :n], gsb[:, :], inb[:, q:q + n],
                                 start=False, stop=True, skip_group_check=True)
                act_out(out8, pb, q, n, AF.Prelu, bias=cold(cb_col), alpha=ALPHA)
            prev = None
            for (q, n) in qt:
                pa = psA.tile([128, 2 * FC], f32, tag="psA")
                conv_dr(pa, kdsb, in8, q, n)
                if prev is not None:
                    tail(prev)
                t1 = t1p.tile([128, 2 * FC], bf16, tag="t1")
                nc.scalar.activation(t1[:, :n], pa[:, :n], AF.Prelu,
                                     scale=1.0 / KS, bias=cold(BI_Z), alpha=ALPHA)
                prev = (t1, q, n)
            tail(prev)
            halo(out8)

        # ---- network ----
        da_stage(Xb, X8, kd1, cw1, g1, BI_CB1, O1)

        # conv1 -> prelu -> fp8 frame
        for (q, n) in qt:
            pa = psA.tile([128, 2 * FC], f32, tag="psA")
            conv_dr(pa, w1, O1, q, n)
            act_out(O2, pa, q, n, AF.Prelu, scale=1.0 / WS, bias=cold(BI_B1),
                    alpha=ALPHA)
        halo(O2)

        # conv2 (+ additive map as a K=18 pass) -> identity+bias -> bf16+fp8
        O38 = f8pool.tile([128, FF], f8, tag="f8")
        for (q, n) in qt:
            pa = psA.tile([128, 2 * FC], f32, tag="psA")
            for p in range(5):
                rhs, np_ = dr_rhs(O2, q, n, PAIRS[p])
                nc.tensor.matmul(pa[:, :np_], w2[:, p, :, :], rhs,
                                 start=(p == 0), stop=False, perf_mode=DR,
                                 skip_group_check=True)
            nc.tensor.matmul(pa[:, :n], wa[:, :], af[:, q:q + n],
                             start=False, stop=True, skip_group_check=True)
            act_out(O3b, pa, q, n, AF.Identity, scale=1.0 / WS, bias=cold(BI_B2))
            nc.vector.tensor_copy(O38[:, q:q + n], O3b[:, q:q + n])
        halo(O3b)
        halo(O38)

        O4 = f8pool.tile([128, FF], f8, tag="f8")
        da_stage(O3b, O38, kd2, cw2, g2, BI_CB2, O4)

        # ---- conv3 + residual: x (bf16, scaled by WS via diag weights)
        # and WS*b3 accumulate straight into PSUM; Act drains with 1/WS ----
        for j, (q, n) in enumerate(qt):
            pa = psA.tile([128, 2 * FC], f32, tag="psA")
            conv_dr(pa, w3, O4, q, n)
            nc.tensor.matmul(pa[:, :n], rid[:, :], Xb[:, q:q + n],
                             start=False, stop=False, skip_group_check=True)
            nc.tensor.matmul(pa[:, :n], b3r[:, :], ones[:, :n],
                             start=False, stop=True, skip_group_check=True)
            ot = outp.tile([128, 2, 192], f32, tag="ot")
            nc.scalar.activation(
                ot[:, :, :],
                pa[:, :n].rearrange("p (a b) -> p a b", b=FC)[:, :, 0:192],
                AF.Identity, scale=1.0 / WS)
            r0 = q // FC - 1  # image row of the pair
            qs[j % 3].dma_start(
                out=y_d[:, r0 * 192:(r0 + 2) * 192]
                .rearrange("p (r c) -> p r c", c=192),
                in_=ot[0:64, :, :])
            qs[(j + 1) % 3].dma_start(
                out=y_d[:, (96 + r0) * 192:(96 + r0 + 2) * 192]
                .rearrange("p (r c) -> p r c", c=192),
                in_=ot[64:128, :, :])

    nc.compile()
    return nc


def _pad_frame(xb, dtype):
    """(64,192,192) fp32 -> (128, FR*FC) dual-half padded frame."""
    fr = np.zeros((128, FR, FC), np.float32)
    fr[0:64, 1:97, 2:194] = xb[:, 0:96, :]
    fr[0:64, 97, 2:194] = xb[:, 96, :]
    fr[64:128, 1:97, 2:194] = xb[:, 96:192, :]
    fr[64:128, 0, 2:194] = xb[:, 95, :]
    return np.ascontiguousarray(fr.reshape(128, FF)).astype(dtype)


def _leaky_np(v):
    return np.where(v >= 0, v, ALPHA * v)


def _host_precompute(x, d, p):
    """Build per-core input maps. p: dict of raw weight arrays."""
    d = d.astype(np.float64)
    kern = {}
    att = {}
    for i in (1, 2):
        kw1, kw2 = p[f'da{i}_kw1'].astype(np.float64), p[f'da{i}_kw2'].astype(np.float64)
        ca1, ca2 = p[f'da{i}_ca1'].astype(np.float64), p[f'da{i}_ca2'].astype(np.float64)
        kern[i] = _leaky_np(d @ kw1.T) @ kw2.T          # (B, 576) [c*9+t]
        z = _leaky_np(d @ ca1.T) @ ca2.T
        att[i] = 1.0 / (1.0 + np.exp(-z))               # (B, 64)
    a32 = _leaky_np(d @ p['add_w1'].astype(np.float64).T) @ \
        p['add_w2'].astype(np.float64).T                # (B, 1024)

    cidx = np.arange(128) % 64
    hidx = np.arange(128) // 64

    def convw_dr(w):
        # (O, C, 3, 3) fp32 -> [128, 5, 2, 128] f8 block-diag DoubleRow taps
        wq = (w.astype(np.float32) * WS).astype(F8).astype(np.float32)
        wt = wq.transpose(1, 2, 3, 0).reshape(64, 9, 64)  # [c, t, o]
        out = np.zeros((128, 5, 2, 128), np.float32)
        for pi, (ta, tb) in enumerate(PAIRS):
            blk = np.zeros((64, 2, 64), np.float32)
            blk[:, 0, :] = wt[:, ta, :]
            if tb is not None:
                blk[:, 1, :] = wt[:, tb, :]
            out[0:64, pi, :, 0:64] = blk
            out[64:128, pi, :, 64:128] = blk
        return np.ascontiguousarray(out).astype(F8)

    def cw_bd(w):
        # (O, C) -> [128, 128] bf16 block-diag: [p, o]
        out = np.zeros((128, 128), np.float32)
        out[0:64, 0:64] = w.T
        out[64:128, 64:128] = w.T
        return np.ascontiguousarray(out).astype(BF16)

    w1 = convw_dr(p['conv1_w'])
    w2 = convw_dr(p['conv2_w'])
    w3 = convw_dr(p['conv3_w'])
    cw1 = cw_bd(p['da1_cw'])
    cw2 = cw_bd(p['da2_cw'])

    # additive-map conv weights: wa[(h,t), o_col] = WS * sum_c conv2_w[o,c,t]
    w2sum = p['conv2_w'].astype(np.float64).sum(axis=1).reshape(64, 9)  # [o, t]
    wa = np.zeros((18, 128), np.float32)
    for h in range(2):
        for t in range(9):
            wa[h * 9 + t, h * 64:(h + 1) * 64] = WS * w2sum[:, t]
    wa = np.ascontiguousarray(wa).astype(BF16)

    rid = np.ascontiguousarray(_diag128(np.full(128, WS, np.float32))).astype(BF16)
    b3r = np.ascontiguousarray(
        (WS * p['conv3_b'].astype(np.float32)[cidx]).reshape(1, 128)).astype(BF16)

    maps = []
    for b in range(B):
        kd = {}
        for i in (1, 2):
            kc = (kern[i][b].reshape(64, 9).astype(np.float32) * KS) \
                .astype(F8).astype(np.float32)           # [c, t]
            kdl = np.zeros((128, 5, 2, 128), np.float32)
            for pi, (ta, tb) in enumerate(PAIRS):
                kdl[np.arange(128), pi, 0, np.arange(128)] = kc[cidx, ta]
                if tb is not None:
                    kdl[np.arange(128), pi, 1, np.arange(128)] = kc[cidx, tb]
            kd[i] = np.ascontiguousarray(kdl).astype(F8)
        g = {i: np.ascontiguousarray(_diag128(att[i][b][cidx])).astype(BF16)
             for i in (1, 2)}
        bias = np.zeros((128, 5), np.float32)
        bias[:, BI_B1] = p['conv1_b'][cidx]
        bias[:, BI_B2] = p['conv2_b'][cidx]
        bias[:, BI_CB1] = p['da1_cb'][cidx]
        bias[:, BI_CB2] = p['da2_cb'][cidx]

        # additive map frames: 18 partitions = 2 halves x 9 tap shifts
        a = a32[b].astype(np.float32).reshape(32, 32)
        aup = a[np.arange(192) // 6][:, np.arange(192) // 6]  # (192,192)
        afr = np.zeros((2, FF), np.float32)
        fr0 = np.zeros((FR, FC), np.float32)
        fr0[1:97, 2:194] = aup[0:96]
        fr0[97, 2:194] = aup[96]
        afr[0] = fr0.reshape(FF)
        fr1 = np.zeros((FR, FC), np.float32)
        fr1[1:97, 2:194] = aup[96:192]
        fr1[0, 2:194] = aup[95]
        afr[1] = fr1.reshape(FF)
        af = np.zeros((18, FF), np.float32)
        for h in range(2):
            for t in range(9):
                dlt = _delta(t)
                src = afr[h]
                dst = np.zeros(FF, np.float32)
                if dlt >= 0:
                    dst[:FF - dlt] = src[dlt:]
                else:
                    dst[-dlt:] = src[:FF + dlt]
                af[h * 9 + t] = dst
        maps.append(dict(
            xb=_pad_frame(x[b], BF16),
            x8=_pad_frame(x[b], F8),
            rid=rid, b3r=b3r,
            w1=w1, w2=w2, w3=w3, kd1=kd[1], kd2=kd[2], cw1=cw1, cw2=cw2,
            g1=g[1], g2=g[2], wa=wa,
            af=np.ascontiguousarray(af).astype(BF16),
            bias=bias))
    return maps


def _diag128(v):
    out = np.zeros((128, 128), np.float32)
    out[np.arange(128), np.arange(128)] = v
    return out


def kernel(**inputs):
    from concourse.bass_utils import run_bass_kernel_spmd

    x = np.asarray(inputs['x'], np.float32)
    d = np.asarray(inputs['d'], np.float32)
    in_maps = _host_precompute(x, d, inputs)

    if 'nc' not in _CACHE:
        _CACHE['nc'] = _build_nc()
    nc = _CACHE['nc']

    try:
        res = run_bass_kernel_spmd(nc, in_maps, list(range(B)))
    except Exception:
        # transient NRT_EXEC_UNIT_UNRECOVERABLE observed on back-to-back
        # runs; a single retry is free and often clears it
        res = run_bass_kernel_spmd(nc, in_maps, list(range(B)))
    out = np.stack([np.asarray(res.results[i]['y'], np.float32).reshape(C, H, W)
                    for i in range(B)])
    return out
